# revision 16
# baseline (speedup 1.0000x reference)
"""Trainium2 Bass kernel for nn_PositiveWaveFunction (tanh-RNN + linear head + softmax).

Math (per reference):
  x      = [init(1,0); data[:S-1]]                    # shifted inputs  [S,B,2]
  pre_t  = W_ih @ x_t + (b_ih + b_hh)                 # input projection
  h_t    = tanh(pre_t + W_hh @ h_{t-1}),  h_0 = 0     # 1024 sequential steps
  logits = lin_W @ h_t + lin_b                        # [S,B,2]
  out    = softmax(logits, axis=2)

Trick: softmax over 2 classes == sigmoid of the logit difference:
  d_t   = (lin_W[0]-lin_W[1]) . h_t + (lin_b[0]-lin_b[1])
  out0  = sigmoid(d_t), out1 = 1 - out0
so the device only produces d_t (an M=1 matmul); sigmoid runs on host.

Sharding: data-parallel over batch, B=256 -> 32 per core on 8 cores.
Per-core layout: h kept as [H=2x128 partitions, batch=32 free] fp32 in a
129-slot SBUF ring.  Per step: 4 accumulating 128x128 matmuls on PE into a
PSUM group tile that was pre-filled (16 steps at a time, N=512 matmuls) with
the input projection + biases, then ONE fused tanh on ACT reading both
128-row halves straight from PSUM and writing the ring.  The d-projection
runs per 16-step group on PE (lhsT = wdiff replicated x4), DVE copies the
PSUM result to an SBUF staging buffer, one DMA at the end.
"""

import sys

sys.path.insert(0, "/opt/trn_rl_repo")

import numpy as np
import ml_dtypes

import concourse.bass as bass  # noqa: F401
import concourse.tile as tile
from concourse import bacc, mybir
from concourse.bass_utils import run_bass_kernel_spmd

S = 1024
B = 256
I = 2
H = 256
NCORES = 8
BL = B // NCORES  # 32 batch per core
GT = 16  # steps per psum group
G = S // GT  # 64 groups
RING = 129  # h history ring slots (8 groups + 1)

F32 = mybir.dt.float32
BF16 = mybir.dt.bfloat16
AF = mybir.ActivationFunctionType
BF16_NP = ml_dtypes.bfloat16

_CACHE = {}


def _slot_runs(g):
    """Contiguous ring-slot runs covering steps t = g*GT+1 .. g*GT+GT."""
    slots = [(g * GT + 1 + i) % RING for i in range(GT)]
    runs = [[slots[0], 1]]
    for s in slots[1:]:
        if s == runs[-1][0] + runs[-1][1]:
            runs[-1][1] += 1
        else:
            runs.append([s, 1])
    return runs


def build_nc(n_groups=G, do_scan=True, do_tanh=True, do_dproj=True, reps=1):
    nc = bacc.Bacc("TRN2", target_bir_lowering=False, debug=False, num_devices=NCORES)

    xa_d = nc.dram_tensor("xa", [2, S, BL], BF16, kind="ExternalInput")
    a2_d = nc.dram_tensor("a2", [2, 2, 128], BF16, kind="ExternalInput")
    bt_d = nc.dram_tensor("bt", [1, 2, 128], BF16, kind="ExternalInput")
    w_d = nc.dram_tensor("w", [128, 2, 2, 128], F32, kind="ExternalInput")
    wd_d = nc.dram_tensor("wd", [128, 2, 1], F32, kind="ExternalInput")
    od_d = nc.dram_tensor("od", [1, S * BL], F32, kind="ExternalOutput")

    with tile.TileContext(nc) as tc:
        with (
            tc.tile_pool(name="sg", bufs=1) as sg,
            tc.tile_pool(name="psp", bufs=2, space="PSUM") as psp,
            tc.tile_pool(name="dpsp", bufs=2, space="PSUM") as dpsp,
            tc.tile_pool(name="odp", bufs=2) as odp,
        ):
            OD_GROUPS = 8  # groups per staging chunk
            OD_CHUNK = OD_GROUPS * GT * BL  # 4096 floats
            xa = sg.tile([2, S, BL], BF16)
            a2 = sg.tile([2, 2, 128], BF16)
            bt = sg.tile([1, 2, 128], BF16)
            w = sg.tile([128, 2, 2, 128], F32)
            wd = sg.tile([128, 2, 1], F32)
            ones = sg.tile([1, GT * BL], BF16)
            hr = sg.tile([128, 2, RING, BL], F32)

            nc.sync.dma_start(out=xa[:], in_=xa_d.ap())
            nc.sync.dma_start(out=a2[:], in_=a2_d.ap())
            nc.sync.dma_start(out=bt[:], in_=bt_d.ap())
            nc.sync.dma_start(out=w[:], in_=w_d.ap())
            nc.sync.dma_start(out=wd[:], in_=wd_d.ap())
            nc.vector.memset(ones[:], 1.0)

            ps_tiles = {}
            od_state = {}

            def prefill(g):
                t = psp.tile([128, 2, GT, BL], F32)
                ps_tiles[g] = t
                for jh in range(2):
                    nc.tensor.matmul(
                        t[:, jh],
                        a2[:, jh],
                        xa[:, g * GT : (g + 1) * GT, :],
                        start=True,
                        stop=False,
                        skip_group_check=True,
                    )
                    nc.tensor.matmul(
                        t[:, jh],
                        bt[:, jh],
                        ones[:],
                        start=False,
                        stop=False,
                        skip_group_check=True,
                    )

            def dproj(g):
                dp = dpsp.tile([1, GT * BL], F32)
                col = 0
                runs = _slot_runs(g)
                for ri, (s0, ln) in enumerate(runs):
                    for kt in range(2):
                        nc.tensor.matmul(
                            dp[:, col * BL : (col + ln) * BL],
                            wd[:, kt, :],
                            hr[:, kt, s0 : s0 + ln, :],
                            start=(kt == 0),
                            stop=(kt == 1),
                            skip_group_check=True,
                        )
                    col += ln
                gc = g % OD_GROUPS
                if gc == 0:
                    od_state["tile"] = odp.tile(
                        [1, OD_CHUNK], F32, name=f"odc{g}", tag="odc"
                    )
                nc.vector.tensor_copy(
                    out=od_state["tile"][:, gc * GT * BL : (gc + 1) * GT * BL],
                    in_=dp[:],
                )
                if gc == OD_GROUPS - 1:
                    chunk = g // OD_GROUPS
                    nc.sync.dma_start(
                        out=od_d.ap()[:, chunk * OD_CHUNK : (chunk + 1) * OD_CHUNK],
                        in_=od_state["tile"][:],
                    )

            def body():
                nc.vector.memset(hr[:, :, 0, :], 0.0)
                prefill(0)
                for g in range(n_groups):
                    emit_group(g)
                if not do_dproj or n_groups % OD_GROUPS != 0:
                    # ensure od output is written so the NEFF has the output
                    odt = odp.tile([1, OD_CHUNK], F32, name="odfill", tag="odc")
                    nc.vector.memset(odt[:], 0.0)
                    nc.sync.dma_start(
                        out=od_d.ap()[:, 0:OD_CHUNK],
                        in_=odt[:],
                    )

            def emit_group(g):
                pst = ps_tiles.pop(g)
                for tl in range(GT):
                    t_ = g * GT + tl + 1
                    sl_prev = (t_ - 1) % RING
                    sl = t_ % RING
                    if do_scan:
                        for jh in range(2):
                            for kt in range(2):
                                nc.tensor.matmul(
                                    pst[:, jh, tl, :],
                                    w[:, kt, jh, :],
                                    hr[:, kt, sl_prev, :],
                                    start=False,
                                    stop=(kt == 1),
                                    skip_group_check=True,
                                )
                    if do_tanh:
                        nc.scalar.activation(
                            out=hr[:, :, sl, :],
                            in_=pst[:, :, tl, :],
                            func=AF.Tanh,
                        )
                if g + 1 < n_groups:
                    prefill(g + 1)
                if do_dproj:
                    dproj(g)

            if reps == 1:
                body()
            else:
                with tc.For_i(0, reps, 1):
                    body()

    nc.compile()
    return nc


def _get_nc():
    if "nc" not in _CACHE:
        _CACHE["nc"] = build_nc()
    return _CACHE["nc"]


def _prep_inputs(data, W_ih, b_ih, W_hh, b_hh, lin_W, lin_b):
    data = np.asarray(data, dtype=np.float32)
    W_ih = np.asarray(W_ih, dtype=np.float32)
    b_ih = np.asarray(b_ih, dtype=np.float32)
    W_hh = np.asarray(W_hh, dtype=np.float32)
    b_hh = np.asarray(b_hh, dtype=np.float32)
    lin_W = np.asarray(lin_W, dtype=np.float32)
    lin_b = np.asarray(lin_b, dtype=np.float32)

    # shifted input sequence
    x_shift = np.empty((S, B, I), dtype=np.float32)
    x_shift[0] = np.array([1.0, 0.0], dtype=np.float32)
    x_shift[1:] = data[: S - 1]

    a2 = np.ascontiguousarray(W_ih.T.reshape(2, 2, 128)).astype(BF16_NP)
    bt = (b_ih + b_hh).reshape(1, 2, 128).astype(BF16_NP)
    # w[kp, kt, jh, jc] = W_hh[jh*128+jc, kt*128+kp]
    wsb = np.ascontiguousarray(
        W_hh.T.reshape(2, 128, 2, 128).transpose(1, 0, 2, 3)
    ).astype(np.float32)
    wdiff = lin_W[0] - lin_W[1]  # [256]
    wdv = np.ascontiguousarray(wdiff.reshape(2, 128).T[:, :, None]).astype(np.float32)
    cdiff = float(lin_b[0] - lin_b[1])

    in_maps = []
    for c in range(NCORES):
        xa = np.ascontiguousarray(
            x_shift[:, c * BL : (c + 1) * BL, :].transpose(2, 0, 1)
        ).astype(BF16_NP)
        in_maps.append({"xa": xa, "a2": a2, "bt": bt, "w": wsb, "wd": wdv})
    return in_maps, cdiff


def _postprocess(results, cdiff):
    probs = np.empty((S, B, I), dtype=np.float32)
    for c in range(NCORES):
        od = np.asarray(results[c]["od"], dtype=np.float64)
        # od[0, (t-1)*BL + b]
        d = od.reshape(S, BL)
        p0 = 1.0 / (1.0 + np.exp(-(d + cdiff)))
        probs[:, c * BL : (c + 1) * BL, 0] = p0.astype(np.float32)
        probs[:, c * BL : (c + 1) * BL, 1] = (1.0 - p0).astype(np.float32)
    return probs


def run(trace=False, **inputs):
    """Run the kernel; returns (probs, BassKernelResults)."""
    nc = _get_nc()
    in_maps, cdiff = _prep_inputs(**inputs)
    res = run_bass_kernel_spmd(nc, in_maps, core_ids=list(range(NCORES)), trace=trace)
    return _postprocess(res.results, cdiff), res


def kernel(**inputs):
    probs, _ = run(trace=False, **inputs)
    return probs


# revision 24
# speedup vs baseline: 3.3576x; 3.3576x over previous
"""Trainium2 Bass kernel for nn_PositiveWaveFunction (tanh-RNN + linear head + softmax).

Math (per reference):
  x      = [init(1,0); data[:S-1]]                    # shifted inputs  [S,B,2]
  pre_t  = W_ih @ x_t + (b_ih + b_hh)                 # input projection
  h_t    = tanh(pre_t + W_hh @ h_{t-1}),  h_0 = 0     # 1024 sequential steps
  logits = lin_W @ h_t + lin_b                        # [S,B,2]
  out    = softmax(logits, axis=2)

Trick: softmax over 2 classes == sigmoid of the logit difference:
  d_t   = (lin_W[0]-lin_W[1]) . h_t + (lin_b[0]-lin_b[1])
  out0  = sigmoid(d_t), out1 = 1 - out0
so the device only produces d_t (an M=1 matmul); sigmoid runs on host.

Sharding: data-parallel over batch, B=256 -> 32 per core on 8 cores.
Per-core layout: h kept as [H=2x128 partitions, batch=32 free] fp32 in a
129-slot SBUF ring.  Per step: 4 accumulating 128x128 matmuls on PE into a
PSUM group tile that was pre-filled (16 steps at a time, N=512 matmuls) with
the input projection + biases, then ONE fused tanh on ACT reading both
128-row halves straight from PSUM and writing the ring.  The d-projection
runs per 16-step group on PE (lhsT = wdiff replicated x4), DVE copies the
PSUM result to an SBUF staging buffer, one DMA at the end.
"""

import sys

sys.path.insert(0, "/opt/trn_rl_repo")

import numpy as np
import ml_dtypes

import concourse.bass as bass  # noqa: F401
import concourse.tile as tile
from concourse import bacc, mybir
from concourse.bass_utils import run_bass_kernel_spmd

S = 1024
B = 256
I = 2
H = 256
NCORES = 8
BL = B // NCORES  # 32 batch per core
GT = 16  # steps per psum group
G = S // GT  # 64 groups
RING = 129  # h history ring slots (8 groups + 1)

F32 = mybir.dt.float32
BF16 = mybir.dt.bfloat16
FP16 = mybir.dt.float16
AF = mybir.ActivationFunctionType
BF16_NP = ml_dtypes.bfloat16

_CACHE = {}


def _slot_runs(g):
    """Contiguous ring-slot runs covering steps t = g*GT+1 .. g*GT+GT."""
    slots = [(g * GT + 1 + i) % RING for i in range(GT)]
    runs = [[slots[0], 1]]
    for s in slots[1:]:
        if s == runs[-1][0] + runs[-1][1]:
            runs[-1][1] += 1
        else:
            runs.append([s, 1])
    return runs


def build_nc(n_groups=G, do_scan=True, do_tanh=True, do_dproj=True, reps=1):
    nc = bacc.Bacc("TRN2", target_bir_lowering=False, debug=False, num_devices=NCORES)

    xa_d = nc.dram_tensor("xa", [2, S, BL], FP16, kind="ExternalInput")
    a2_d = nc.dram_tensor("a2", [2, 2, 128], FP16, kind="ExternalInput")
    bt_d = nc.dram_tensor("bt", [1, 2, 128], FP16, kind="ExternalInput")
    w_d = nc.dram_tensor("w", [128, 2, 2, 128], FP16, kind="ExternalInput")
    wd_d = nc.dram_tensor("wd", [128, 2, 1], FP16, kind="ExternalInput")
    od_d = nc.dram_tensor("od", [1, S * BL], F32, kind="ExternalOutput")

    with tile.TileContext(nc) as tc:
        with (
            tc.tile_pool(name="sg", bufs=1) as sg,
            tc.tile_pool(name="psp", bufs=2, space="PSUM") as psp,
            tc.tile_pool(name="dpsp", bufs=2, space="PSUM") as dpsp,
            tc.tile_pool(name="odp", bufs=2) as odp,
        ):
            OD_GROUPS = 8  # groups per staging chunk
            OD_CHUNK = OD_GROUPS * GT * BL  # 4096 floats
            xa = sg.tile([2, S, BL], FP16)
            a2 = sg.tile([2, 2, 128], FP16)
            bt = sg.tile([1, 2, 128], FP16)
            w = sg.tile([128, 2, 2, 128], FP16)
            wd = sg.tile([128, 2, 1], FP16)
            ones = sg.tile([1, GT * BL], FP16)
            hr = sg.tile([128, 2, RING, BL], FP16)

            nc.sync.dma_start(out=xa[:], in_=xa_d.ap())
            nc.sync.dma_start(out=a2[:], in_=a2_d.ap())
            nc.sync.dma_start(out=bt[:], in_=bt_d.ap())
            nc.sync.dma_start(out=w[:], in_=w_d.ap())
            nc.sync.dma_start(out=wd[:], in_=wd_d.ap())
            nc.vector.memset(ones[:], 1.0)

            ps_tiles = {}
            dp_tiles = {}
            od_state = {}

            def prefill_a2(g, jh):
                if jh == 0:
                    ps_tiles[g] = psp.tile(
                        [128, 2, GT, BL], F32, name=f"ps{g}", tag="ps"
                    )
                nc.tensor.matmul(
                    ps_tiles[g][:, jh],
                    a2[:, jh],
                    xa[:, g * GT : (g + 1) * GT, :],
                    start=True,
                    stop=False,
                    skip_group_check=True,
                )

            def prefill_bias(g, jh):
                nc.tensor.matmul(
                    ps_tiles[g][:, jh],
                    bt[:, jh],
                    ones[:],
                    start=False,
                    stop=False,
                    skip_group_check=True,
                )

            def dproj_mm(g, kt):
                if kt == 0:
                    dp_tiles[g] = dpsp.tile(
                        [1, GT * BL], F32, name=f"dp{g}", tag="dp"
                    )
                dp = dp_tiles[g]
                col = 0
                for ri, (s0, ln) in enumerate(_slot_runs(g)):
                    # start=True clears has_written for the WHOLE psum bank, so
                    # it must only be set on the very first matmul into this dp
                    # tile; later run-splits overwrite-where-clear on their own
                    # columns via start=False.
                    nc.tensor.matmul(
                        dp[:, col * BL : (col + ln) * BL],
                        wd[:, kt, :],
                        hr[:, kt, s0 : s0 + ln, :],
                        start=(kt == 0 and ri == 0),
                        stop=(kt == 1),
                        skip_group_check=True,
                    )
                    col += ln

            def dproj_copy(g):
                dp = dp_tiles.pop(g)
                gc = g % OD_GROUPS
                if gc == 0:
                    od_state["tile"] = odp.tile(
                        [1, OD_CHUNK], F32, name=f"odc{g}", tag="odc"
                    )
                nc.vector.tensor_copy(
                    out=od_state["tile"][:, gc * GT * BL : (gc + 1) * GT * BL],
                    in_=dp[:],
                )
                if gc == OD_GROUPS - 1:
                    chunk = g // OD_GROUPS
                    nc.sync.dma_start(
                        out=od_d.ap()[:, chunk * OD_CHUNK : (chunk + 1) * OD_CHUNK],
                        in_=od_state["tile"][:],
                    )

            def emit_group(g):
                pst = ps_tiles.pop(g)
                for tl in range(GT):
                    t_ = g * GT + tl + 1
                    sl_prev = (t_ - 1) % RING
                    sl = t_ % RING
                    if do_scan:
                        for jh in range(2):
                            for kt in range(2):
                                nc.tensor.matmul(
                                    pst[:, jh, tl, :],
                                    w[:, kt, jh, :],
                                    hr[:, kt, sl_prev, :],
                                    start=False,
                                    stop=(kt == 1),
                                    skip_group_check=True,
                                )
                    if do_tanh:
                        nc.scalar.activation(
                            out=hr[:, :, sl, :],
                            in_=pst[:, :, tl, :],
                            func=AF.Tanh,
                        )
                    # spread prefill of g+1 / dproj of g-1 into step gaps
                    if g + 1 < n_groups:
                        if tl == 0:
                            prefill_a2(g + 1, 0)
                        elif tl == 2:
                            prefill_bias(g + 1, 0)
                        elif tl == 4:
                            prefill_a2(g + 1, 1)
                        elif tl == 6:
                            prefill_bias(g + 1, 1)
                    if do_dproj and g > 0:
                        if tl == 8:
                            dproj_mm(g - 1, 0)
                        elif tl == 10:
                            dproj_mm(g - 1, 1)
                        elif tl == 12:
                            dproj_copy(g - 1)

            def body():
                nc.vector.memset(hr[:, :, 0, :], 0.0)
                prefill_a2(0, 0)
                prefill_bias(0, 0)
                prefill_a2(0, 1)
                prefill_bias(0, 1)
                for g in range(n_groups):
                    emit_group(g)
                if do_dproj:
                    dproj_mm(n_groups - 1, 0)
                    dproj_mm(n_groups - 1, 1)
                    dproj_copy(n_groups - 1)
                if not do_dproj or n_groups % OD_GROUPS != 0:
                    # ensure od output is written so the NEFF has the output
                    odt = odp.tile([1, OD_CHUNK], F32, name="odfill", tag="odc")
                    nc.vector.memset(odt[:], 0.0)
                    nc.sync.dma_start(
                        out=od_d.ap()[:, 0:OD_CHUNK],
                        in_=odt[:],
                    )

            if reps == 1:
                body()
            else:
                with tc.For_i(0, reps, 1):
                    body()

    nc.compile()
    return nc


def _get_nc():
    if "nc" not in _CACHE:
        _CACHE["nc"] = build_nc()
    return _CACHE["nc"]


def _prep_inputs(data, W_ih, b_ih, W_hh, b_hh, lin_W, lin_b):
    data = np.asarray(data, dtype=np.float32)
    W_ih = np.asarray(W_ih, dtype=np.float32)
    b_ih = np.asarray(b_ih, dtype=np.float32)
    W_hh = np.asarray(W_hh, dtype=np.float32)
    b_hh = np.asarray(b_hh, dtype=np.float32)
    lin_W = np.asarray(lin_W, dtype=np.float32)
    lin_b = np.asarray(lin_b, dtype=np.float32)

    # shifted input sequence
    x_shift = np.empty((S, B, I), dtype=np.float32)
    x_shift[0] = np.array([1.0, 0.0], dtype=np.float32)
    x_shift[1:] = data[: S - 1]

    a2 = np.ascontiguousarray(W_ih.T.reshape(2, 2, 128)).astype(np.float16)
    bt = (b_ih + b_hh).reshape(1, 2, 128).astype(np.float16)
    # w[kp, kt, jh, jc] = W_hh[jh*128+jc, kt*128+kp]
    wsb = np.ascontiguousarray(
        W_hh.T.reshape(2, 128, 2, 128).transpose(1, 0, 2, 3)
    ).astype(np.float16)
    wdiff = lin_W[0] - lin_W[1]  # [256]
    wdv = np.ascontiguousarray(wdiff.reshape(2, 128).T[:, :, None]).astype(np.float16)
    cdiff = float(lin_b[0] - lin_b[1])

    in_maps = []
    for c in range(NCORES):
        xa = np.ascontiguousarray(
            x_shift[:, c * BL : (c + 1) * BL, :].transpose(2, 0, 1)
        ).astype(np.float16)
        in_maps.append({"xa": xa, "a2": a2, "bt": bt, "w": wsb, "wd": wdv})
    return in_maps, cdiff


def _postprocess(results, cdiff):
    probs = np.empty((S, B, I), dtype=np.float32)
    for c in range(NCORES):
        od = np.asarray(results[c]["od"], dtype=np.float64)
        # od[0, (t-1)*BL + b]
        d = od.reshape(S, BL)
        p0 = 1.0 / (1.0 + np.exp(-(d + cdiff)))
        probs[:, c * BL : (c + 1) * BL, 0] = p0.astype(np.float32)
        probs[:, c * BL : (c + 1) * BL, 1] = (1.0 - p0).astype(np.float32)
    return probs


def run(trace=False, **inputs):
    """Run the kernel; returns (probs, BassKernelResults)."""
    nc = _get_nc()
    in_maps, cdiff = _prep_inputs(**inputs)
    res = run_bass_kernel_spmd(nc, in_maps, core_ids=list(range(NCORES)), trace=trace)
    return _postprocess(res.results, cdiff), res


def kernel(**inputs):
    probs, _ = run(trace=False, **inputs)
    return probs


# revision 31
# speedup vs baseline: 3.9739x; 1.1835x over previous
"""Trainium2 Bass kernel for nn_PositiveWaveFunction (tanh-RNN + linear head + softmax).

Math (per reference):
  x      = [init(1,0); data[:S-1]]                    # shifted inputs  [S,B,2]
  pre_t  = W_ih @ x_t + (b_ih + b_hh)                 # input projection
  h_t    = tanh(pre_t + W_hh @ h_{t-1}),  h_0 = 0     # 1024 sequential steps
  logits = lin_W @ h_t + lin_b                        # [S,B,2]
  out    = softmax(logits, axis=2)

Trick: softmax over 2 classes == sigmoid of the logit difference:
  d_t   = (lin_W[0]-lin_W[1]) . h_t + (lin_b[0]-lin_b[1])
  out0  = sigmoid(d_t), out1 = 1 - out0
so the device only produces d_t (an M=1 matmul); sigmoid runs on host.

Sharding: data-parallel over batch, B=256 -> 32 per core on 8 cores.
Per-core layout: h kept as [H=2x128 partitions, batch=32 free] fp32 in a
129-slot SBUF ring.  Per step: 4 accumulating 128x128 matmuls on PE into a
PSUM group tile that was pre-filled (16 steps at a time, N=512 matmuls) with
the input projection + biases, then ONE fused tanh on ACT reading both
128-row halves straight from PSUM and writing the ring.  The d-projection
runs per 16-step group on PE (lhsT = wdiff replicated x4), DVE copies the
PSUM result to an SBUF staging buffer, one DMA at the end.
"""

import sys

sys.path.insert(0, "/opt/trn_rl_repo")

import numpy as np
import ml_dtypes

import concourse.bass as bass  # noqa: F401
import concourse.tile as tile
from concourse import bacc, mybir
from concourse.bass_utils import run_bass_kernel_spmd

S = 1024
B = 256
I = 2
H = 256
NCORES = 8
BL = B // NCORES  # 32 batch per core
GT = 16  # steps per psum group
G = S // GT  # 64 groups
RING = 129  # h history ring slots (8 groups + 1)

F32 = mybir.dt.float32
BF16 = mybir.dt.bfloat16
FP16 = mybir.dt.float16
AF = mybir.ActivationFunctionType
BF16_NP = ml_dtypes.bfloat16

_CACHE = {}


def _slot_runs(g):
    """Contiguous ring-slot runs covering steps t = g*GT+1 .. g*GT+GT."""
    slots = [(g * GT + 1 + i) % RING for i in range(GT)]
    runs = [[slots[0], 1]]
    for s in slots[1:]:
        if s == runs[-1][0] + runs[-1][1]:
            runs[-1][1] += 1
        else:
            runs.append([s, 1])
    return runs


def build_nc(n_groups=G, do_scan=True, do_tanh=True, do_dproj=True, reps=1):
    nc = bacc.Bacc("TRN2", target_bir_lowering=False, debug=False, num_devices=NCORES)

    xa_d = nc.dram_tensor("xa", [3, S, BL], FP16, kind="ExternalInput")
    a2_d = nc.dram_tensor("a2", [3, 2, 128], FP16, kind="ExternalInput")
    w_d = nc.dram_tensor("w", [128, 2, 2, 128], FP16, kind="ExternalInput")
    wd_d = nc.dram_tensor("wd", [128, 2, 1], FP16, kind="ExternalInput")
    od_d = nc.dram_tensor("od", [1, S * BL], F32, kind="ExternalOutput")

    with tile.TileContext(nc) as tc:
        with (
            tc.tile_pool(name="sg", bufs=1) as sg,
            tc.tile_pool(name="psp", bufs=2, space="PSUM") as psp,
            tc.tile_pool(name="dpsp", bufs=2, space="PSUM") as dpsp,
            tc.tile_pool(name="odp", bufs=2) as odp,
        ):
            OD_GROUPS = 8  # groups per staging chunk
            OD_CHUNK = OD_GROUPS * GT * BL  # 4096 floats
            xa = sg.tile([3, S, BL], FP16)
            a2 = sg.tile([3, 2, 128], FP16)
            w = sg.tile([128, 2, 2, 128], FP16)
            wd = sg.tile([128, 2, 1], FP16)
            hr = sg.tile([128, 2, RING, BL], FP16)

            nc.sync.dma_start(out=xa[:], in_=xa_d.ap())
            nc.sync.dma_start(out=a2[:], in_=a2_d.ap())
            nc.sync.dma_start(out=w[:], in_=w_d.ap())
            nc.sync.dma_start(out=wd[:], in_=wd_d.ap())

            ps_tiles = {}
            dp_tiles = {}
            od_state = {}

            HGT = GT // 2

            def prefill_mm(g, jh, half):
                if jh == 0 and half == 0:
                    ps_tiles[g] = psp.tile(
                        [128, 2, GT, BL], F32, name=f"ps{g}", tag="ps"
                    )
                # half==0 start=True clears the whole jh bank (pending-zero);
                # half==1 uses start=False and overwrites-where-clear.
                nc.tensor.matmul(
                    ps_tiles[g][:, jh, half * HGT : (half + 1) * HGT, :],
                    a2[:, jh],
                    xa[:, g * GT + half * HGT : g * GT + (half + 1) * HGT, :],
                    start=(half == 0),
                    stop=False,
                    skip_group_check=True,
                )

            def dproj_mm(g, kt):
                if kt == 0:
                    dp_tiles[g] = dpsp.tile(
                        [1, GT * BL], F32, name=f"dp{g}", tag="dp"
                    )
                dp = dp_tiles[g]
                col = 0
                for ri, (s0, ln) in enumerate(_slot_runs(g)):
                    # start=True clears has_written for the WHOLE psum bank, so
                    # it must only be set on the very first matmul into this dp
                    # tile; later run-splits overwrite-where-clear on their own
                    # columns via start=False.
                    nc.tensor.matmul(
                        dp[:, col * BL : (col + ln) * BL],
                        wd[:, kt, :],
                        hr[:, kt, s0 : s0 + ln, :],
                        start=(kt == 0 and ri == 0),
                        stop=(kt == 1),
                        skip_group_check=True,
                    )
                    col += ln

            def dproj_copy(g):
                dp = dp_tiles.pop(g)
                gc = g % OD_GROUPS
                if gc == 0:
                    od_state["tile"] = odp.tile(
                        [1, OD_CHUNK], F32, name=f"odc{g}", tag="odc"
                    )
                nc.vector.tensor_copy(
                    out=od_state["tile"][:, gc * GT * BL : (gc + 1) * GT * BL],
                    in_=dp[:],
                )
                if gc == OD_GROUPS - 1:
                    chunk = g // OD_GROUPS
                    nc.sync.dma_start(
                        out=od_d.ap()[:, chunk * OD_CHUNK : (chunk + 1) * OD_CHUNK],
                        in_=od_state["tile"][:],
                    )

            def emit_group(g):
                pst = ps_tiles.pop(g)
                for tl in range(GT):
                    t_ = g * GT + tl + 1
                    sl_prev = (t_ - 1) % RING
                    sl = t_ % RING
                    if do_scan:
                        for jh in range(2):
                            for kt in range(2):
                                nc.tensor.matmul(
                                    pst[:, jh, tl, :],
                                    w[:, kt, jh, :],
                                    hr[:, kt, sl_prev, :],
                                    start=False,
                                    stop=(kt == 1),
                                    skip_group_check=True,
                                )
                    if do_tanh:
                        nc.scalar.activation(
                            out=hr[:, :, sl, :],
                            in_=pst[:, :, tl, :],
                            func=AF.Tanh,
                        )
                    # spread prefill of g+1 / dproj of g-1 into step gaps
                    if g + 1 < n_groups:
                        if tl == 0:
                            prefill_mm(g + 1, 0, 0)
                        elif tl == 2:
                            prefill_mm(g + 1, 0, 1)
                        elif tl == 4:
                            prefill_mm(g + 1, 1, 0)
                        elif tl == 6:
                            prefill_mm(g + 1, 1, 1)
                    if do_dproj and g > 0:
                        if tl == 8:
                            dproj_mm(g - 1, 0)
                        elif tl == 10:
                            dproj_mm(g - 1, 1)
                        elif tl == 12:
                            dproj_copy(g - 1)

            def body():
                nc.vector.memset(hr[:, :, 0, :], 0.0)
                for jh in range(2):
                    for half in range(2):
                        prefill_mm(0, jh, half)
                for g in range(n_groups):
                    emit_group(g)
                if do_dproj:
                    dproj_mm(n_groups - 1, 0)
                    dproj_mm(n_groups - 1, 1)
                    dproj_copy(n_groups - 1)
                if not do_dproj or n_groups % OD_GROUPS != 0:
                    # ensure od output is written so the NEFF has the output
                    odt = odp.tile([1, OD_CHUNK], F32, name="odfill", tag="odc")
                    nc.vector.memset(odt[:], 0.0)
                    nc.sync.dma_start(
                        out=od_d.ap()[:, 0:OD_CHUNK],
                        in_=odt[:],
                    )

            if reps == 1:
                body()
            else:
                with tc.For_i(0, reps, 1):
                    body()

    nc.compile()
    return nc


def _get_nc():
    if "nc" not in _CACHE:
        _CACHE["nc"] = build_nc()
    return _CACHE["nc"]


def _prep_inputs(data, W_ih, b_ih, W_hh, b_hh, lin_W, lin_b):
    data = np.asarray(data, dtype=np.float32)
    W_ih = np.asarray(W_ih, dtype=np.float32)
    b_ih = np.asarray(b_ih, dtype=np.float32)
    W_hh = np.asarray(W_hh, dtype=np.float32)
    b_hh = np.asarray(b_hh, dtype=np.float32)
    lin_W = np.asarray(lin_W, dtype=np.float32)
    lin_b = np.asarray(lin_b, dtype=np.float32)

    # shifted input sequence, with a ones-row appended for the bias
    x_shift = np.empty((S, B, I), dtype=np.float32)
    x_shift[0] = np.array([1.0, 0.0], dtype=np.float32)
    x_shift[1:] = data[: S - 1]

    a2 = np.empty((3, 2, 128), dtype=np.float16)
    a2[0:2] = W_ih.T.reshape(2, 2, 128)
    a2[2] = (b_ih + b_hh).reshape(2, 128)
    # w[kp, kt, jh, jc] = W_hh[jh*128+jc, kt*128+kp]
    wsb = np.ascontiguousarray(
        W_hh.T.reshape(2, 128, 2, 128).transpose(1, 0, 2, 3)
    ).astype(np.float16)
    wdiff = lin_W[0] - lin_W[1]  # [256]
    wdv = np.ascontiguousarray(wdiff.reshape(2, 128).T[:, :, None]).astype(np.float16)
    cdiff = float(lin_b[0] - lin_b[1])

    in_maps = []
    for c in range(NCORES):
        xa = np.ones((3, S, BL), dtype=np.float16)
        xa[0:2] = x_shift[:, c * BL : (c + 1) * BL, :].transpose(2, 0, 1)
        in_maps.append({"xa": xa, "a2": a2, "w": wsb, "wd": wdv})
    return in_maps, cdiff


def _postprocess(results, cdiff):
    probs = np.empty((S, B, I), dtype=np.float32)
    for c in range(NCORES):
        od = np.asarray(results[c]["od"], dtype=np.float64)
        # od[0, (t-1)*BL + b]
        d = od.reshape(S, BL)
        p0 = 1.0 / (1.0 + np.exp(-(d + cdiff)))
        probs[:, c * BL : (c + 1) * BL, 0] = p0.astype(np.float32)
        probs[:, c * BL : (c + 1) * BL, 1] = (1.0 - p0).astype(np.float32)
    return probs


def run(trace=False, **inputs):
    """Run the kernel; returns (probs, BassKernelResults)."""
    nc = _get_nc()
    in_maps, cdiff = _prep_inputs(**inputs)
    res = run_bass_kernel_spmd(nc, in_maps, core_ids=list(range(NCORES)), trace=trace)
    return _postprocess(res.results, cdiff), res


def kernel(**inputs):
    probs, _ = run(trace=False, **inputs)
    return probs
